# revision 41
# baseline (speedup 1.0000x reference)
"""Trainium2 Bass kernel for the pLDDT loss (nn_PlddtLoss).

Reference computation (see problem):
  - pairwise distances d_true/d_pred over [B=4, L=2048] coords
  - lDDT pair scores from 4 |d_pred-d_true| thresholds, masked to
    d_true < 15, j != i
  - per-residue true_score -> bin index -> NLL of plddt_logits under
    log_softmax, averaged over valid residues (scalar output).

Sharding: 8 cores = 4 batches x 2 row-halves. Each core computes the
full [1024 rows x 2048 cols] pair block for its shard plus the
per-residue NLL, and returns [loss_sum, valid_count]; the host sums the
8 pairs and divides (the only host-side math).

Device algorithm per core:
  d^2(i,j) = n_i + n_j - 2<x_i, x_j> is produced directly by one K=5
  matmul per tile (rows: -2*x^T, n_i, ones against x^T, ones,
  n_j + 1e6*(1-mask_j)); the 1e6 penalty pushes masked j outside the
  15A radius so masking costs nothing. The diagonal contributes exactly
  (score 1, valid 1) and is removed by subtracting (mask_i > 0) from
  both row sums. Threshold counts use a fused scalar_tensor_tensor
  chain; row reductions ride the free accum_out ports. All row sums are
  exact in fp32 (quarter/integer values), so the bin index reproduces
  the reference's floor(100*(n/d)*0.5) decisions via an fp32 divide.
"""

import os
from contextlib import ExitStack

import ml_dtypes
import numpy as np

import concourse.bass as bass
import concourse.tile as tile
from concourse import mybir
from concourse.bass_utils import run_bass_kernel_spmd

F32 = mybir.dt.float32
BF16 = mybir.dt.bfloat16
F16 = mybir.dt.float16
OP = mybir.AluOpType
AF = mybir.ActivationFunctionType
NPBF = ml_dtypes.bfloat16

B = 4
L = 2048
NBINS = 50
P = 128
HALF = L // 2          # rows per core
NT = HALF // P         # row tiles per core
NCH = L // 512         # 512-wide matmul chunks per row tile
KAUG = 25              # augmented contraction rows (bf16 3-term split)
# d^2 -> sqrt bias; must exceed PE fp32 accumulation error on the diagonal
# (partial sums reach ~4e3 -> a few ulp ~ 5e-4). The bias shifts dt and dp
# together, so it cancels to first order in dist_error = |dp - dt|.
RADIUS2_BIAS = 2e-3

LAST_RESULTS = None
_NC_CACHE = None


def split_excess_waits(nc, maxw=1):
    """walrus CTRL codegen rejects instructions with several sem waits;
    move excess waits onto same-engine no-op carriers placed just before."""
    k = 0
    for f in nc.m.functions:
        for blk in f.blocks:
            out = []
            for inst in blk.instructions:
                si = inst.sync_info
                if si is not None and si.on_wait and len(si.on_wait) > maxw:
                    waits = list(si.on_wait)
                    while len(waits) > maxw:
                        chunk, waits = waits[:maxw], waits[maxw:]
                        nop = mybir.InstNoOp(name=f"waitnop-{k}", ins=[], outs=[])
                        k += 1
                        nop.engine = inst.engine
                        nop.sync_info = mybir.SyncInfo(on_wait=chunk, on_update=[])
                        out.append(nop)
                    si.on_wait = waits
                out.append(inst)
            blk.instructions[:] = out
    return k


def build_nc(split_waits=True):
    nc = bass.Bass()
    # Augmented bf16 matmul operands (built host-side during sharding, see
    # _aug_operands): A.T @ B = n_i + n_j - 2<x~_i,x~_j> (+penalty) = d^2.
    Amt = nc.declare_dram_parameter("Amt", [KAUG, HALF], BF16, isOutput=False)
    Bmt = nc.declare_dram_parameter("Bmt", [KAUG, L], BF16, isOutput=False)
    Amp = nc.declare_dram_parameter("Amp", [KAUG, HALF], BF16, isOutput=False)
    Bmp = nc.declare_dram_parameter("Bmp", [KAUG, L], BF16, isOutput=False)
    mcols = nc.declare_dram_parameter("mcols", [P, NT], F32, isOutput=False)
    lg = nc.declare_dram_parameter("lg", [HALF, NBINS], F32, isOutput=False)
    iob = nc.declare_dram_parameter("iob", [2, NBINS], F32, isOutput=False)
    out = nc.declare_dram_parameter("out", [1, 2], F32, isOutput=True)

    with tile.TileContext(nc) as tc, ExitStack() as ctx:
        singles = ctx.enter_context(tc.tile_pool(name="singles", bufs=1))
        work2 = ctx.enter_context(tc.tile_pool(name="work2", bufs=2))
        work1 = ctx.enter_context(tc.tile_pool(name="work1", bufs=1))
        psum = ctx.enter_context(tc.tile_pool(name="psum", bufs=1, space="PSUM"))

        Bt = singles.tile([KAUG, L], BF16)
        Bp = singles.tile([KAUG, L], BF16)
        At = singles.tile([KAUG, HALF], BF16)
        Ap = singles.tile([KAUG, HALF], BF16)
        nc.sync.dma_start(out=Bt, in_=Bmt[:, :])
        nc.sync.dma_start(out=Bp, in_=Bmp[:, :])
        nc.sync.dma_start(out=At, in_=Amt[:, :])
        nc.sync.dma_start(out=Ap, in_=Amp[:, :])
        mcols_t = singles.tile([P, NT], F32)
        nc.sync.dma_start(out=mcols_t, in_=mcols[:, :])
        lgt = singles.tile([P, NT, NBINS], F32)
        nc.sync.dma_start(out=lgt, in_=lg.rearrange("(t p) c -> p t c", p=P))

        # ---- per-row accumulators ----
        # scols[k][:, t] = sum_j (ae' < c_k)  (ae' carries the +32768
        # radius/mask penalty, so these are the *valid-masked* counts)
        scols = [singles.tile([P, NT], F32, tag=f"scol{k}", name=f"scol{k}")
                 for k in range(4)]
        dacc = singles.tile([P, NT], F32)   # sum_j valid  (= ae' < 16384)
        sqbias = singles.tile([P, 1], F32)
        nc.vector.memset(sqbias, RADIUS2_BIAS)
        pbias = singles.tile([P, 1], F32)
        nc.vector.memset(pbias, -15.0 * 2.0 ** 35)

        # ---- pair loop over row tiles ----
        for t in range(NT):
            pt = psum.tile([P, L], F32, tag="pt")
            pp = psum.tile([P, L], F32, tag="pp")
            for c in range(NCH):
                cs = slice(c * 512, (c + 1) * 512)
                nc.tensor.matmul(pt[:, cs], At[:, t * P:(t + 1) * P], Bt[:, cs],
                                 start=True, stop=True)
            for c in range(NCH):
                cs = slice(c * 512, (c + 1) * 512)
                nc.tensor.matmul(pp[:, cs], Ap[:, t * P:(t + 1) * P], Bp[:, cs],
                                 start=True, stop=True)

            dt = work2.tile([P, L], F32, tag="dt")
            nc.scalar.activation(dt, pt, AF.Sqrt, bias=sqbias)
            dp = work2.tile([P, L], F32, tag="dp")
            nc.scalar.activation(dp, pp, AF.Sqrt, bias=sqbias)
            # pen = Relu((dt-15)*2^35): 0 for dt<15, else >= 32768 (the
            # smallest fp32 step above 15 is ~9.5e-7 -> 32768). Power-of-2
            # scale keeps the 15.0 cut exact. Also fires for masked j,
            # whose dt is ~1000 via the matmul penalty row.
            pen = work1.tile([P, L], F16, tag="pen")
            nc.scalar.activation(pen, dt, AF.Relu, bias=pbias,
                                 scale=float(2.0 ** 35))
            e = work2.tile([P, L], F16, tag="e")
            nc.vector.tensor_sub(e, dp, dt)
            q2 = work2.tile([P, L], F16, tag="q2")
            nc.vector.tensor_mul(q2, e, e)
            q2p = work2.tile([P, L], F16, tag="ae")
            nc.vector.tensor_add(q2p, q2, pen)
            # (q2p < c^2) == (|e| < c AND valid); 16384 recovers `valid`
            for k, c in enumerate((0.25, 1.0, 4.0, 16.0, 16384.0)):
                acc = dacc if k == 4 else scols[k]
                sk = work1.tile([P, L], F16, tag=f"s{k % 2}", name=f"sk{k}")
                nc.vector.tensor_scalar(sk, q2p, c, None, OP.is_lt, OP.add,
                                        accum_out=acc[:, t:t + 1])

        # ---- per-residue stats ([P, NT], exact fp32 integers/quarters) ----
        m01 = singles.tile([P, NT], F32)
        nc.vector.tensor_scalar(m01, mcols_t, 0.0, None, OP.is_gt)
        t1 = singles.tile([P, NT], F32)
        nc.vector.tensor_add(t1, scols[0], scols[1])
        t2 = singles.tile([P, NT], F32)
        nc.vector.tensor_add(t2, scols[2], scols[3])
        nc.vector.tensor_add(t1, t1, t2)
        numer = singles.tile([P, NT], F32)
        # numer = 0.25*sum_k scols[k] - m01  (drop the diagonal)
        nc.vector.scalar_tensor_tensor(numer, t1, 0.25, m01, OP.mult,
                                       OP.subtract)
        den = singles.tile([P, NT], F32)
        nc.vector.tensor_sub(den, dacc, m01)
        denc = singles.tile([P, NT], F32)
        nc.vector.tensor_scalar(denc, den, 1.0, None, OP.max)
        # 50*numer and max(den,1) are exact fp32 integers/quarters, so the
        # bin window test (c*d <= 50n < (c+1)*d) is exact and provably equal
        # to the reference's floor(fp32(100*fp32(n/d))*0.5) decisions.
        n50 = singles.tile([P, NT], F32)
        nc.vector.tensor_scalar(n50, numer, 50.0, None, OP.mult)
        rv = singles.tile([P, NT], F32)
        nc.vector.scalar_tensor_tensor(rv, den, 0.0, m01, OP.is_gt, OP.mult)

        # ---- log-softmax NLL at the selected bin ----
        mcol = singles.tile([P, NT], F32)
        nc.vector.tensor_reduce(mcol, lgt, mybir.AxisListType.X, OP.max)
        negm = singles.tile([P, NT], F32)
        nc.vector.tensor_scalar(negm, mcol, -1.0, None, OP.mult)
        et = singles.tile([P, NT, NBINS], F32)
        sume = singles.tile([P, NT], F32)
        for t in range(NT):
            nc.scalar.activation(et[:, t, :], lgt[:, t, :], AF.Exp,
                                 bias=negm[:, t:t + 1],
                                 accum_out=sume[:, t:t + 1])
        lse = singles.tile([P, NT], F32)
        nc.scalar.activation(lse, sume, AF.Ln)
        lzm = singles.tile([P, NT], F32)
        nc.vector.tensor_add(lzm, lse, mcol)     # logsumexp = m + log(sum exp)

        iol = singles.tile([P, NBINS], F32)   # [0, 1, ..., 49]
        ioh = singles.tile([P, NBINS], F32)   # [1, ..., 49, 3e8] (open top bin)
        iob_ap = iob.ap()
        nc.sync.dma_start(out=iol, in_=bass.AP(
            tensor=iob_ap.tensor, offset=0, ap=[[0, P]] + iob_ap.ap[1:]))
        nc.sync.dma_start(out=ioh, in_=bass.AP(
            tensor=iob_ap.tensor, offset=NBINS, ap=[[0, P]] + iob_ap.ap[1:]))
        xs_ = singles.tile([P, NT], F32)
        for t in range(NT):
            a = work1.tile([P, NBINS], F32, tag="u")
            nc.vector.tensor_scalar(a, iol, denc[:, t:t + 1], None, OP.mult)
            w1 = work1.tile([P, NBINS], F32, tag="w1")
            nc.vector.tensor_scalar(w1, a, n50[:, t:t + 1], None, OP.is_le)
            bb = work1.tile([P, NBINS], F32, tag="bb")
            nc.vector.tensor_scalar(bb, ioh, denc[:, t:t + 1], None, OP.mult)
            w2 = work1.tile([P, NBINS], F32, tag="w2")
            nc.vector.scalar_tensor_tensor(w2, bb, n50[:, t:t + 1], w1,
                                           OP.is_gt, OP.mult)
            jk2 = work1.tile([P, NBINS], F32, tag="jk2")
            # out = (1.0*w2)*logits ; accum_out = sum -> x[bin]
            nc.vector.scalar_tensor_tensor(jk2, w2, 1.0, lgt[:, t, :],
                                           OP.mult, OP.mult,
                                           accum_out=xs_[:, t:t + 1])

        pl = singles.tile([P, NT], F32)
        nc.vector.tensor_sub(pl, lzm, xs_)       # -logp[bin] = logZ - x[bin]
        lo = singles.tile([P, NT], F32)
        nc.vector.tensor_mul(lo, pl, rv)

        red = singles.tile([P, 2], F32)
        nc.vector.tensor_reduce(red[:, 0:1], lo, mybir.AxisListType.X, OP.add)
        nc.vector.tensor_reduce(red[:, 1:2], rv, mybir.AxisListType.X, OP.add)
        ones = singles.tile([P, 1], F32)
        nc.vector.memset(ones, 1.0)
        ps = psum.tile([1, 2], F32, tag="pt")   # reuse a freed pair-psum slot
        nc.tensor.matmul(ps, ones, red, start=True, stop=True)
        res = singles.tile([1, 2], F32)
        nc.vector.tensor_copy(res, ps)
        nc.sync.dma_start(out=out[:, :], in_=res)

    if split_waits:
        split_excess_waits(nc)
    return nc


def _get_nc():
    global _NC_CACHE
    if _NC_CACHE is None:
        _NC_CACHE = build_nc()
    return _NC_CACHE


def _iob_const():
    iob = np.empty((2, NBINS), np.float32)
    iob[0] = np.arange(NBINS)
    iob[1] = np.arange(1, NBINS + 1)
    iob[1, -1] = 3e8     # top bin absorbs everything (the reference clip)
    return iob


def _bf16_split3(v):
    """v (fp32/fp64) -> three bf16 terms summing to v to ~2^-26 rel."""
    a = v.astype(NPBF)
    r = v - a.astype(np.float64)
    b = r.astype(NPBF)
    c = (r - b.astype(np.float64)).astype(NPBF)
    return a, b, c


def _aug_operands(x, rows, penalty_mask=None):
    """Host-side build of the augmented [KAUG, *] bf16 matmul operands.

    x is decomposed as h+m+l (bf16 terms); bf16 products are exact on the
    PE and accumulate in fp32, so A.T @ B = n_i + n_j - 2<x~_i, x~_j>
    (+1e6 mask penalty) at fp32-level accuracy but 4x fp32 matmul speed.
    Large-magnitude rows come first so fp32 accumulation rounding happens
    before the small correction terms land."""
    h, m, low = _bf16_split3(x.astype(np.float64))
    xt_ = h.astype(np.float64) + m.astype(np.float64) + low.astype(np.float64)
    n = (xt_ ** 2).sum(-1)
    n1, n2, n3 = _bf16_split3(n)

    nr = rows.stop - rows.start
    A = np.zeros((KAUG, nr), NPBF)
    Bm = np.zeros((KAUG, x.shape[0]), NPBF)
    hT, mT, lT = h.T, m.T, low.T
    h2 = (-2.0 * h.astype(np.float32)).astype(NPBF).T
    m2 = (-2.0 * m.astype(np.float32)).astype(NPBF).T
    l2 = (-2.0 * low.astype(np.float32)).astype(NPBF).T

    A[0] = n1[rows]; A[1] = n2[rows]; A[2] = n3[rows]
    A[3:6] = h2[:, rows]
    A[6:10] = 1.0
    A[10:13] = h2[:, rows]
    A[13:16] = m2[:, rows]
    A[16:19] = h2[:, rows]
    A[19:22] = l2[:, rows]
    A[22:25] = m2[:, rows]

    Bm[0:3] = 1.0
    Bm[3:6] = hT
    Bm[6] = n1; Bm[7] = n2; Bm[8] = n3
    if penalty_mask is not None:
        Bm[9] = (np.float32(1e6) * (1.0 - penalty_mask)).astype(NPBF)
    Bm[10:13] = mT
    Bm[13:16] = hT
    Bm[16:19] = lT
    Bm[19:22] = hT
    Bm[22:25] = mT
    return np.ascontiguousarray(A), np.ascontiguousarray(Bm)


def make_in_maps(plddt_logits, x_pred, x_true, mask):
    lgf = np.ascontiguousarray(np.asarray(plddt_logits, np.float32))
    xp = np.asarray(x_pred, np.float32)
    xt = np.asarray(x_true, np.float32)
    mk = np.asarray(mask, np.float32)
    in_maps = []
    for core in range(8):
        b, h = divmod(core, 2)
        r0 = h * HALF
        rows = slice(r0, r0 + HALF)
        Amt, Bmt = _aug_operands(xt[b], rows, penalty_mask=mk[b])
        Amp, Bmp = _aug_operands(xp[b], rows)
        in_maps.append({
            "Amt": Amt, "Bmt": Bmt, "Amp": Amp, "Bmp": Bmp,
            "mcols": np.ascontiguousarray(mk[b, rows].reshape(NT, P).T),
            "lg": np.ascontiguousarray(lgf[b, rows]),
            "iob": _iob_const(),
        })
    return in_maps


def _ensure_ntff_hook():
    """Best-effort registration of the axon NTFF profiling hook so that
    trace=True works; returns True when tracing is usable."""
    try:
        from antenv.axon_hooks import get_axon_ntff_profile_hook  # noqa: F401
        return True
    except ImportError:
        pass
    try:
        import sys
        import types

        from trn_agent_boot.trn_boot import _ntff_profile_via_ctypes

        so_path = os.environ.get("PJRT_LIBRARY_PATH", "/opt/axon/libaxon_pjrt.so")
        hook = _ntff_profile_via_ctypes(so_path)
        mod = types.ModuleType("antenv.axon_hooks")
        state = {"hook": hook}
        mod.set_axon_ntff_profile_hook = lambda h: state.__setitem__("hook", h)
        mod.get_axon_ntff_profile_hook = lambda: state["hook"]
        sys.modules["antenv.axon_hooks"] = mod
        import antenv
        antenv.axon_hooks = mod
        return hook is not None
    except Exception:
        return False


def kernel(plddt_logits, x_pred, x_true, mask):
    global LAST_RESULTS
    in_maps = make_in_maps(plddt_logits, x_pred, x_true, mask)
    nc = _get_nc()
    trace = bool(os.environ.get("PLDDT_TRACE")) and _ensure_ntff_hook()
    br = run_bass_kernel_spmd(
        nc, in_maps, list(range(8)),
        trace=trace,
    )
    LAST_RESULTS = br
    tot_l = 0.0
    tot_c = 0.0
    for r in br.results:
        o = np.asarray(r["out"]).reshape(2)
        tot_l += float(o[0])
        tot_c += float(o[1])
    return np.float32(tot_l / max(tot_c, 1.0))


# revision 47
# speedup vs baseline: 1.2758x; 1.2758x over previous
"""Trainium2 Bass kernel for the pLDDT loss (nn_PlddtLoss).

Reference computation (see problem):
  - pairwise distances d_true/d_pred over [B=4, L=2048] coords
  - lDDT pair scores from 4 |d_pred-d_true| thresholds, masked to
    d_true < 15, j != i
  - per-residue true_score -> bin index -> NLL of plddt_logits under
    log_softmax, averaged over valid residues (scalar output).

Sharding: 8 cores = 4 batches x 2 row-halves. Each core computes the
full [1024 rows x 2048 cols] pair block for its shard plus the
per-residue NLL, and returns [loss_sum, valid_count]; the host sums the
8 pairs and divides (the only host-side math).

Device algorithm per core:
  d^2(i,j) = n_i + n_j - 2<x_i, x_j> is produced directly by one K=5
  matmul per tile (rows: -2*x^T, n_i, ones against x^T, ones,
  n_j + 1e6*(1-mask_j)); the 1e6 penalty pushes masked j outside the
  15A radius so masking costs nothing. The diagonal contributes exactly
  (score 1, valid 1) and is removed by subtracting (mask_i > 0) from
  both row sums. Threshold counts use a fused scalar_tensor_tensor
  chain; row reductions ride the free accum_out ports. All row sums are
  exact in fp32 (quarter/integer values), so the bin index reproduces
  the reference's floor(100*(n/d)*0.5) decisions via an fp32 divide.
"""

import os
from contextlib import ExitStack

import ml_dtypes
import numpy as np

import concourse.bass as bass
import concourse.tile as tile
from concourse import mybir
from concourse.bass_utils import run_bass_kernel_spmd

F32 = mybir.dt.float32
BF16 = mybir.dt.bfloat16
F16 = mybir.dt.float16
I16 = mybir.dt.int16
OP = mybir.AluOpType
AF = mybir.ActivationFunctionType
NPBF = ml_dtypes.bfloat16

B = 4
L = 2048
NBINS = 50
P = 128
HALF = L // 2          # rows per core
NT = HALF // P         # row tiles per core
NCH = L // 512         # 512-wide matmul chunks per row tile
KAUG = 25              # augmented contraction rows (bf16 3-term split)
# d^2 -> sqrt bias; must exceed PE fp32 accumulation error on the diagonal
# (partial sums reach ~4e3 -> a few ulp ~ 5e-4). The bias shifts dt and dp
# together, so it cancels to first order in dist_error = |dp - dt|.
RADIUS2_BIAS = 2e-3

LAST_RESULTS = None
_NC_CACHE = None


def split_excess_waits(nc, maxw=1):
    """walrus CTRL codegen rejects instructions with several sem waits;
    move excess waits onto same-engine no-op carriers placed just before."""
    k = 0
    for f in nc.m.functions:
        for blk in f.blocks:
            out = []
            for inst in blk.instructions:
                si = inst.sync_info
                if si is not None and si.on_wait and len(si.on_wait) > maxw:
                    waits = list(si.on_wait)
                    while len(waits) > maxw:
                        chunk, waits = waits[:maxw], waits[maxw:]
                        nop = mybir.InstNoOp(name=f"waitnop-{k}", ins=[], outs=[])
                        k += 1
                        nop.engine = inst.engine
                        nop.sync_info = mybir.SyncInfo(on_wait=chunk, on_update=[])
                        out.append(nop)
                    si.on_wait = waits
                out.append(inst)
            blk.instructions[:] = out
    return k


def build_nc(split_waits=True):
    nc = bass.Bass()
    # Augmented bf16 matmul operands (built host-side during sharding, see
    # _aug_operands): A.T @ B = n_i + n_j - 2<x~_i,x~_j> (+penalty) = d^2.
    Amt = nc.declare_dram_parameter("Amt", [KAUG, HALF], BF16, isOutput=False)
    Bmt = nc.declare_dram_parameter("Bmt", [KAUG, L], BF16, isOutput=False)
    Amp = nc.declare_dram_parameter("Amp", [KAUG, HALF], BF16, isOutput=False)
    Bmp = nc.declare_dram_parameter("Bmp", [KAUG, L], BF16, isOutput=False)
    mcols = nc.declare_dram_parameter("mcols", [P, NT], F32, isOutput=False)
    lg = nc.declare_dram_parameter("lg", [HALF, NBINS], F32, isOutput=False)
    iob = nc.declare_dram_parameter("iob", [2, NBINS], F32, isOutput=False)
    out = nc.declare_dram_parameter("out", [1, 2], F32, isOutput=True)

    with tile.TileContext(nc) as tc, ExitStack() as ctx:
        singles = ctx.enter_context(tc.tile_pool(name="singles", bufs=1))
        work2 = ctx.enter_context(tc.tile_pool(name="work2", bufs=2))
        work1 = ctx.enter_context(tc.tile_pool(name="work1", bufs=1))
        psum = ctx.enter_context(tc.tile_pool(name="psum", bufs=1, space="PSUM"))

        Bt = singles.tile([KAUG, L], BF16)
        Bp = singles.tile([KAUG, L], BF16)
        At = singles.tile([KAUG, HALF], BF16)
        Ap = singles.tile([KAUG, HALF], BF16)
        nc.sync.dma_start(out=Bt, in_=Bmt[:, :])
        nc.sync.dma_start(out=Bp, in_=Bmp[:, :])
        nc.sync.dma_start(out=At, in_=Amt[:, :])
        nc.sync.dma_start(out=Ap, in_=Amp[:, :])
        mcols_t = singles.tile([P, NT], F32)
        nc.sync.dma_start(out=mcols_t, in_=mcols[:, :])
        lgt = singles.tile([P, NT, NBINS], F32)
        nc.sync.dma_start(out=lgt, in_=lg.rearrange("(t p) c -> p t c", p=P))

        # ---- per-row accumulators ----
        nacc4 = singles.tile([P, NT], F32)  # sum_j score4 (0..4 per pair)
        dacc = singles.tile([P, NT], F32)   # sum_j valid
        sqbias = singles.tile([P, 1], F32)
        nc.vector.memset(sqbias, RADIUS2_BIAS)
        pbias = singles.tile([P, 1], F32)
        nc.vector.memset(pbias, -15.0 * 2.0 ** 35)

        # ---- pair loop over row tiles ----
        for t in range(NT):
            pt = psum.tile([P, L], F32, tag="pt")
            pp = psum.tile([P, L], F32, tag="pp")
            for c in range(NCH):
                cs = slice(c * 512, (c + 1) * 512)
                nc.tensor.matmul(pt[:, cs], At[:, t * P:(t + 1) * P], Bt[:, cs],
                                 start=True, stop=True)
            for c in range(NCH):
                cs = slice(c * 512, (c + 1) * 512)
                nc.tensor.matmul(pp[:, cs], Ap[:, t * P:(t + 1) * P], Bp[:, cs],
                                 start=True, stop=True)

            dt = work2.tile([P, L], F32, tag="dt")
            nc.scalar.activation(dt, pt, AF.Sqrt, bias=sqbias)
            dp = work2.tile([P, L], F32, tag="dp")
            nc.scalar.activation(dp, pp, AF.Sqrt, bias=sqbias)
            # pen = Relu((dt-15)*2^35): 0 for dt<15, else >= 32768 (the
            # smallest fp32 step above 15 is ~9.5e-7 -> 32768). Power-of-2
            # scale keeps the 15.0 cut exact. Also fires for masked j,
            # whose dt is ~1000 via the matmul penalty row.
            pen = work1.tile([P, L], F16, tag="pen")
            nc.scalar.activation(pen, dt, AF.Relu, bias=pbias,
                                 scale=float(2.0 ** 35))
            e = work2.tile([P, L], F16, tag="e")
            nc.vector.tensor_sub(e, dp, dt)
            ep = work2.tile([P, L], F16, tag="ep")
            nc.vector.tensor_add(ep, e, pen)
            q2p = work2.tile([P, L], F16, tag="ae")
            nc.vector.tensor_mul(q2p, ep, ep)
            # q2p = (|dp-dt| + pen)^2 >= 0 in f16. The four thresholds
            # 0.5,1,2,4 on |e| are 2^-2,2^0,2^2,2^4 on e^2 -- pure f16
            # exponent-field tests, so the score is a staircase in the
            # biased exponent u = bits>>10:
            #   score s = clip((20 - u) >> 1, 0, 4)   (exact)
            # and valid = (q2p < 2^14) = (u < 29).
            u = work1.tile([P, L], I16, tag="u16")
            nc.vector.tensor_scalar(u, q2p.bitcast(I16), 10, None,
                                    OP.logical_shift_right)
            uc = work1.tile([P, L], I16, tag="uc16")
            nc.vector.tensor_scalar(uc, u, 12, 20, OP.max, OP.min)
            w = work1.tile([P, L], I16, tag="w16")
            nc.vector.tensor_scalar(w, uc, -1, 20, OP.mult, OP.add)
            s_ = work1.tile([P, L], I16, tag="s16")
            # w in [0, 8] after the clamp, so logical == arithmetic shift
            nc.vector.tensor_scalar(s_, w, 1, None, OP.logical_shift_right)
            sf = work1.tile([P, L], F16, tag="sf")
            nc.vector.tensor_scalar(sf, s_, 1, None, OP.mult, OP.add,
                                    accum_out=nacc4[:, t:t + 1])
            va = work1.tile([P, L], F16, tag="va")
            nc.vector.tensor_scalar(va, u, 29, None, OP.is_lt, OP.add,
                                    accum_out=dacc[:, t:t + 1])

        # ---- per-residue stats ([P, NT], exact fp32 integers/quarters) ----
        m01 = singles.tile([P, NT], F32)
        nc.vector.tensor_scalar(m01, mcols_t, 0.0, None, OP.is_gt)
        numer = singles.tile([P, NT], F32)
        # numer = 0.25*nacc4 - m01  (drop the diagonal)
        nc.vector.scalar_tensor_tensor(numer, nacc4, 0.25, m01, OP.mult,
                                       OP.subtract)
        den = singles.tile([P, NT], F32)
        nc.vector.tensor_sub(den, dacc, m01)
        denc = singles.tile([P, NT], F32)
        nc.vector.tensor_scalar(denc, den, 1.0, None, OP.max)
        # 50*numer and max(den,1) are exact fp32 integers/quarters, so the
        # bin window test (c*d <= 50n < (c+1)*d) is exact and provably equal
        # to the reference's floor(fp32(100*fp32(n/d))*0.5) decisions.
        n50 = singles.tile([P, NT], F32)
        nc.vector.tensor_scalar(n50, numer, 50.0, None, OP.mult)
        rv = singles.tile([P, NT], F32)
        nc.vector.scalar_tensor_tensor(rv, den, 0.0, m01, OP.is_gt, OP.mult)

        # ---- log-softmax NLL at the selected bin ----
        mcol = singles.tile([P, NT], F32)
        nc.vector.tensor_reduce(mcol, lgt, mybir.AxisListType.X, OP.max)
        negm = singles.tile([P, NT], F32)
        nc.vector.tensor_scalar(negm, mcol, -1.0, None, OP.mult)
        et = singles.tile([P, NT, NBINS], F32)
        sume = singles.tile([P, NT], F32)
        for t in range(NT):
            nc.scalar.activation(et[:, t, :], lgt[:, t, :], AF.Exp,
                                 bias=negm[:, t:t + 1],
                                 accum_out=sume[:, t:t + 1])
        lse = singles.tile([P, NT], F32)
        nc.scalar.activation(lse, sume, AF.Ln)
        lzm = singles.tile([P, NT], F32)
        nc.vector.tensor_add(lzm, lse, mcol)     # logsumexp = m + log(sum exp)

        iol = singles.tile([P, NBINS], F32)   # [0, 1, ..., 49]
        ioh = singles.tile([P, NBINS], F32)   # [1, ..., 49, 3e8] (open top bin)
        iob_ap = iob.ap()
        nc.sync.dma_start(out=iol, in_=bass.AP(
            tensor=iob_ap.tensor, offset=0, ap=[[0, P]] + iob_ap.ap[1:]))
        nc.sync.dma_start(out=ioh, in_=bass.AP(
            tensor=iob_ap.tensor, offset=NBINS, ap=[[0, P]] + iob_ap.ap[1:]))
        xs_ = singles.tile([P, NT], F32)
        for t in range(NT):
            a = work1.tile([P, NBINS], F32, tag="u")
            nc.vector.tensor_scalar(a, iol, denc[:, t:t + 1], None, OP.mult)
            w1 = work1.tile([P, NBINS], F32, tag="w1")
            nc.vector.tensor_scalar(w1, a, n50[:, t:t + 1], None, OP.is_le)
            bb = work1.tile([P, NBINS], F32, tag="bb")
            nc.vector.tensor_scalar(bb, ioh, denc[:, t:t + 1], None, OP.mult)
            w2 = work1.tile([P, NBINS], F32, tag="w2")
            nc.vector.scalar_tensor_tensor(w2, bb, n50[:, t:t + 1], w1,
                                           OP.is_gt, OP.mult)
            jk2 = work1.tile([P, NBINS], F32, tag="jk2")
            # out = (1.0*w2)*logits ; accum_out = sum -> x[bin]
            nc.vector.scalar_tensor_tensor(jk2, w2, 1.0, lgt[:, t, :],
                                           OP.mult, OP.mult,
                                           accum_out=xs_[:, t:t + 1])

        pl = singles.tile([P, NT], F32)
        nc.vector.tensor_sub(pl, lzm, xs_)       # -logp[bin] = logZ - x[bin]
        lo = singles.tile([P, NT], F32)
        nc.vector.tensor_mul(lo, pl, rv)

        red = singles.tile([P, 2], F32)
        nc.vector.tensor_reduce(red[:, 0:1], lo, mybir.AxisListType.X, OP.add)
        nc.vector.tensor_reduce(red[:, 1:2], rv, mybir.AxisListType.X, OP.add)
        ones = singles.tile([P, 1], F32)
        nc.vector.memset(ones, 1.0)
        ps = psum.tile([1, 2], F32, tag="pt")   # reuse a freed pair-psum slot
        nc.tensor.matmul(ps, ones, red, start=True, stop=True)
        res = singles.tile([1, 2], F32)
        nc.vector.tensor_copy(res, ps)
        nc.sync.dma_start(out=out[:, :], in_=res)

    if split_waits:
        split_excess_waits(nc)
    return nc


def _get_nc():
    global _NC_CACHE
    if _NC_CACHE is None:
        _NC_CACHE = build_nc()
    return _NC_CACHE


def _iob_const():
    iob = np.empty((2, NBINS), np.float32)
    iob[0] = np.arange(NBINS)
    iob[1] = np.arange(1, NBINS + 1)
    iob[1, -1] = 3e8     # top bin absorbs everything (the reference clip)
    return iob


def _bf16_split3(v):
    """v (fp32/fp64) -> three bf16 terms summing to v to ~2^-26 rel."""
    a = v.astype(NPBF)
    r = v - a.astype(np.float64)
    b = r.astype(NPBF)
    c = (r - b.astype(np.float64)).astype(NPBF)
    return a, b, c


def _aug_operands(x, rows, penalty_mask=None):
    """Host-side build of the augmented [KAUG, *] bf16 matmul operands.

    x is decomposed as h+m+l (bf16 terms); bf16 products are exact on the
    PE and accumulate in fp32, so A.T @ B = n_i + n_j - 2<x~_i, x~_j>
    (+1e6 mask penalty) at fp32-level accuracy but 4x fp32 matmul speed.
    Large-magnitude rows come first so fp32 accumulation rounding happens
    before the small correction terms land."""
    h, m, low = _bf16_split3(x.astype(np.float64))
    xt_ = h.astype(np.float64) + m.astype(np.float64) + low.astype(np.float64)
    n = (xt_ ** 2).sum(-1)
    n1, n2, n3 = _bf16_split3(n)

    nr = rows.stop - rows.start
    A = np.zeros((KAUG, nr), NPBF)
    Bm = np.zeros((KAUG, x.shape[0]), NPBF)
    hT, mT, lT = h.T, m.T, low.T
    h2 = (-2.0 * h.astype(np.float32)).astype(NPBF).T
    m2 = (-2.0 * m.astype(np.float32)).astype(NPBF).T
    l2 = (-2.0 * low.astype(np.float32)).astype(NPBF).T

    A[0] = n1[rows]; A[1] = n2[rows]; A[2] = n3[rows]
    A[3:6] = h2[:, rows]
    A[6:10] = 1.0
    A[10:13] = h2[:, rows]
    A[13:16] = m2[:, rows]
    A[16:19] = h2[:, rows]
    A[19:22] = l2[:, rows]
    A[22:25] = m2[:, rows]

    Bm[0:3] = 1.0
    Bm[3:6] = hT
    Bm[6] = n1; Bm[7] = n2; Bm[8] = n3
    if penalty_mask is not None:
        Bm[9] = (np.float32(1e6) * (1.0 - penalty_mask)).astype(NPBF)
    Bm[10:13] = mT
    Bm[13:16] = hT
    Bm[16:19] = lT
    Bm[19:22] = hT
    Bm[22:25] = mT
    return np.ascontiguousarray(A), np.ascontiguousarray(Bm)


def make_in_maps(plddt_logits, x_pred, x_true, mask):
    lgf = np.ascontiguousarray(np.asarray(plddt_logits, np.float32))
    xp = np.asarray(x_pred, np.float32)
    xt = np.asarray(x_true, np.float32)
    mk = np.asarray(mask, np.float32)
    in_maps = []
    for core in range(8):
        b, h = divmod(core, 2)
        r0 = h * HALF
        rows = slice(r0, r0 + HALF)
        Amt, Bmt = _aug_operands(xt[b], rows, penalty_mask=mk[b])
        Amp, Bmp = _aug_operands(xp[b], rows)
        in_maps.append({
            "Amt": Amt, "Bmt": Bmt, "Amp": Amp, "Bmp": Bmp,
            "mcols": np.ascontiguousarray(mk[b, rows].reshape(NT, P).T),
            "lg": np.ascontiguousarray(lgf[b, rows]),
            "iob": _iob_const(),
        })
    return in_maps


def _ensure_ntff_hook():
    """Best-effort registration of the axon NTFF profiling hook so that
    trace=True works; returns True when tracing is usable."""
    try:
        from antenv.axon_hooks import get_axon_ntff_profile_hook  # noqa: F401
        return True
    except ImportError:
        pass
    try:
        import sys
        import types

        from trn_agent_boot.trn_boot import _ntff_profile_via_ctypes

        so_path = os.environ.get("PJRT_LIBRARY_PATH", "/opt/axon/libaxon_pjrt.so")
        hook = _ntff_profile_via_ctypes(so_path)
        mod = types.ModuleType("antenv.axon_hooks")
        state = {"hook": hook}
        mod.set_axon_ntff_profile_hook = lambda h: state.__setitem__("hook", h)
        mod.get_axon_ntff_profile_hook = lambda: state["hook"]
        sys.modules["antenv.axon_hooks"] = mod
        import antenv
        antenv.axon_hooks = mod
        return hook is not None
    except Exception:
        return False


def kernel(plddt_logits, x_pred, x_true, mask):
    global LAST_RESULTS
    in_maps = make_in_maps(plddt_logits, x_pred, x_true, mask)
    nc = _get_nc()
    trace = bool(os.environ.get("PLDDT_TRACE")) and _ensure_ntff_hook()
    br = run_bass_kernel_spmd(
        nc, in_maps, list(range(8)),
        trace=trace,
    )
    LAST_RESULTS = br
    tot_l = 0.0
    tot_c = 0.0
    for r in br.results:
        o = np.asarray(r["out"]).reshape(2)
        tot_l += float(o[0])
        tot_c += float(o[1])
    return np.float32(tot_l / max(tot_c, 1.0))


# revision 48
# speedup vs baseline: 1.2906x; 1.0116x over previous
"""Trainium2 Bass kernel for the pLDDT loss (nn_PlddtLoss).

Reference computation (see problem):
  - pairwise distances d_true/d_pred over [B=4, L=2048] coords
  - lDDT pair scores from 4 |d_pred-d_true| thresholds, masked to
    d_true < 15, j != i
  - per-residue true_score -> bin index -> NLL of plddt_logits under
    log_softmax, averaged over valid residues (scalar output).

Sharding: 8 cores = 4 batches x 2 row-halves. Each core computes the
full [1024 rows x 2048 cols] pair block for its shard plus the
per-residue NLL, and returns [loss_sum, valid_count]; the host sums the
8 pairs and divides (the only host-side math).

Device algorithm per core:
  d^2(i,j) = n_i + n_j - 2<x_i, x_j> is produced directly by one K=5
  matmul per tile (rows: -2*x^T, n_i, ones against x^T, ones,
  n_j + 1e6*(1-mask_j)); the 1e6 penalty pushes masked j outside the
  15A radius so masking costs nothing. The diagonal contributes exactly
  (score 1, valid 1) and is removed by subtracting (mask_i > 0) from
  both row sums. Threshold counts use a fused scalar_tensor_tensor
  chain; row reductions ride the free accum_out ports. All row sums are
  exact in fp32 (quarter/integer values), so the bin index reproduces
  the reference's floor(100*(n/d)*0.5) decisions via an fp32 divide.
"""

import os
from contextlib import ExitStack

import ml_dtypes
import numpy as np

import concourse.bass as bass
import concourse.tile as tile
from concourse import mybir
from concourse.bass_utils import run_bass_kernel_spmd

F32 = mybir.dt.float32
BF16 = mybir.dt.bfloat16
F16 = mybir.dt.float16
I16 = mybir.dt.int16
OP = mybir.AluOpType
AF = mybir.ActivationFunctionType
NPBF = ml_dtypes.bfloat16

B = 4
L = 2048
NBINS = 50
P = 128
HALF = L // 2          # rows per core
NT = HALF // P         # row tiles per core
NCH = L // 512         # 512-wide matmul chunks per row tile
KAUG = 25              # augmented contraction rows (bf16 3-term split)
# d^2 -> sqrt bias; must exceed PE fp32 accumulation error on the diagonal
# (partial sums reach ~4e3 -> a few ulp ~ 5e-4). The bias shifts dt and dp
# together, so it cancels to first order in dist_error = |dp - dt|.
RADIUS2_BIAS = 2e-3

LAST_RESULTS = None
_NC_CACHE = None


def split_excess_waits(nc, maxw=1):
    """walrus CTRL codegen rejects instructions with several sem waits;
    move excess waits onto same-engine no-op carriers placed just before."""
    k = 0
    for f in nc.m.functions:
        for blk in f.blocks:
            out = []
            for inst in blk.instructions:
                si = inst.sync_info
                if si is not None and si.on_wait and len(si.on_wait) > maxw:
                    waits = list(si.on_wait)
                    while len(waits) > maxw:
                        chunk, waits = waits[:maxw], waits[maxw:]
                        nop = mybir.InstNoOp(name=f"waitnop-{k}", ins=[], outs=[])
                        k += 1
                        nop.engine = inst.engine
                        nop.sync_info = mybir.SyncInfo(on_wait=chunk, on_update=[])
                        out.append(nop)
                    si.on_wait = waits
                out.append(inst)
            blk.instructions[:] = out
    return k


def build_nc(split_waits=True):
    nc = bass.Bass()
    # Augmented bf16 matmul operands (built host-side during sharding, see
    # _aug_operands): A.T @ B = n_i + n_j - 2<x~_i,x~_j> (+penalty) = d^2.
    Amt = nc.declare_dram_parameter("Amt", [KAUG, HALF], BF16, isOutput=False)
    Bmt = nc.declare_dram_parameter("Bmt", [KAUG, L], BF16, isOutput=False)
    Amp = nc.declare_dram_parameter("Amp", [KAUG, HALF], BF16, isOutput=False)
    Bmp = nc.declare_dram_parameter("Bmp", [KAUG, L], BF16, isOutput=False)
    mcols = nc.declare_dram_parameter("mcols", [P, NT], F32, isOutput=False)
    lg = nc.declare_dram_parameter("lg", [HALF, NBINS], F32, isOutput=False)
    iob = nc.declare_dram_parameter("iob", [2, NBINS], F32, isOutput=False)
    out = nc.declare_dram_parameter("out", [1, 2], F32, isOutput=True)

    with tile.TileContext(nc) as tc, ExitStack() as ctx:
        singles = ctx.enter_context(tc.tile_pool(name="singles", bufs=1))
        work2 = ctx.enter_context(tc.tile_pool(name="work2", bufs=3))
        work1 = ctx.enter_context(tc.tile_pool(name="work1", bufs=2))
        psum = ctx.enter_context(tc.tile_pool(name="psum", bufs=1, space="PSUM"))

        Bt = singles.tile([KAUG, L], BF16)
        Bp = singles.tile([KAUG, L], BF16)
        At = singles.tile([KAUG, HALF], BF16)
        Ap = singles.tile([KAUG, HALF], BF16)
        nc.sync.dma_start(out=Bt, in_=Bmt[:, :])
        nc.sync.dma_start(out=Bp, in_=Bmp[:, :])
        nc.sync.dma_start(out=At, in_=Amt[:, :])
        nc.sync.dma_start(out=Ap, in_=Amp[:, :])
        mcols_t = singles.tile([P, NT], F32)
        nc.sync.dma_start(out=mcols_t, in_=mcols[:, :])
        lgt = singles.tile([P, NT, NBINS], F32)
        nc.sync.dma_start(out=lgt, in_=lg.rearrange("(t p) c -> p t c", p=P))

        # ---- per-row accumulators ----
        nacc4 = singles.tile([P, NT], F32)  # sum_j score4 (0..4 per pair)
        dacc = singles.tile([P, NT], F32)   # sum_j valid
        sqbias = singles.tile([P, 1], F32)
        nc.vector.memset(sqbias, RADIUS2_BIAS)
        pbias = singles.tile([P, 1], F32)
        nc.vector.memset(pbias, -15.0 * 2.0 ** 35)

        # ---- pair loop over row tiles ----
        for t in range(NT):
            pt = psum.tile([P, L], F32, tag="pt")
            pp = psum.tile([P, L], F32, tag="pp")
            for c in range(NCH):
                cs = slice(c * 512, (c + 1) * 512)
                nc.tensor.matmul(pt[:, cs], At[:, t * P:(t + 1) * P], Bt[:, cs],
                                 start=True, stop=True)
            for c in range(NCH):
                cs = slice(c * 512, (c + 1) * 512)
                nc.tensor.matmul(pp[:, cs], Ap[:, t * P:(t + 1) * P], Bp[:, cs],
                                 start=True, stop=True)

            dt = work2.tile([P, L], F32, tag="dt")
            nc.scalar.activation(dt, pt, AF.Sqrt, bias=sqbias)
            dp = work2.tile([P, L], F32, tag="dp")
            nc.scalar.activation(dp, pp, AF.Sqrt, bias=sqbias)
            # pen = Relu((dt-15)*2^35): 0 for dt<15, else >= 32768 (the
            # smallest fp32 step above 15 is ~9.5e-7 -> 32768). Power-of-2
            # scale keeps the 15.0 cut exact. Also fires for masked j,
            # whose dt is ~1000 via the matmul penalty row.
            pen = work1.tile([P, L], F16, tag="pen")
            nc.scalar.activation(pen, dt, AF.Relu, bias=pbias,
                                 scale=float(2.0 ** 35))
            e = work2.tile([P, L], F16, tag="e")
            nc.vector.tensor_sub(e, dp, dt)
            ep = work2.tile([P, L], F16, tag="ep")
            nc.vector.tensor_add(ep, e, pen)
            q2p = work2.tile([P, L], F16, tag="ae")
            nc.vector.tensor_mul(q2p, ep, ep)
            # q2p = (|dp-dt| + pen)^2 >= 0 in f16. The four thresholds
            # 0.5,1,2,4 on |e| are 2^-2,2^0,2^2,2^4 on e^2 -- pure f16
            # exponent-field tests, so the score is a staircase in the
            # biased exponent u = bits>>10:
            #   score s = clip((20 - u) >> 1, 0, 4)   (exact)
            # and valid = (q2p < 2^14) = (u < 29).
            u = work1.tile([P, L], I16, tag="u16")
            nc.vector.tensor_scalar(u, q2p.bitcast(I16), 10, None,
                                    OP.logical_shift_right)
            uc = work1.tile([P, L], I16, tag="uc16")
            nc.vector.tensor_scalar(uc, u, 12, 20, OP.max, OP.min)
            w = work1.tile([P, L], I16, tag="w16")
            nc.vector.tensor_scalar(w, uc, -1, 20, OP.mult, OP.add)
            s_ = work1.tile([P, L], I16, tag="s16")
            # w in [0, 8] after the clamp, so logical == arithmetic shift
            nc.vector.tensor_scalar(s_, w, 1, None, OP.logical_shift_right)
            sf = work1.tile([P, L], F16, tag="sf")
            nc.vector.tensor_scalar(sf, s_, 1, None, OP.mult, OP.add,
                                    accum_out=nacc4[:, t:t + 1])
            va = work1.tile([P, L], F16, tag="va")
            nc.vector.tensor_scalar(va, u, 29, None, OP.is_lt, OP.add,
                                    accum_out=dacc[:, t:t + 1])

        # ---- per-residue stats ([P, NT], exact fp32 integers/quarters) ----
        m01 = singles.tile([P, NT], F32)
        nc.vector.tensor_scalar(m01, mcols_t, 0.0, None, OP.is_gt)
        numer = singles.tile([P, NT], F32)
        # numer = 0.25*nacc4 - m01  (drop the diagonal)
        nc.vector.scalar_tensor_tensor(numer, nacc4, 0.25, m01, OP.mult,
                                       OP.subtract)
        den = singles.tile([P, NT], F32)
        nc.vector.tensor_sub(den, dacc, m01)
        denc = singles.tile([P, NT], F32)
        nc.vector.tensor_scalar(denc, den, 1.0, None, OP.max)
        # 50*numer and max(den,1) are exact fp32 integers/quarters, so the
        # bin window test (c*d <= 50n < (c+1)*d) is exact and provably equal
        # to the reference's floor(fp32(100*fp32(n/d))*0.5) decisions.
        n50 = singles.tile([P, NT], F32)
        nc.vector.tensor_scalar(n50, numer, 50.0, None, OP.mult)
        rv = singles.tile([P, NT], F32)
        nc.vector.scalar_tensor_tensor(rv, den, 0.0, m01, OP.is_gt, OP.mult)

        # ---- log-softmax NLL at the selected bin ----
        mcol = singles.tile([P, NT], F32)
        nc.vector.tensor_reduce(mcol, lgt, mybir.AxisListType.X, OP.max)
        negm = singles.tile([P, NT], F32)
        nc.vector.tensor_scalar(negm, mcol, -1.0, None, OP.mult)
        et = singles.tile([P, NT, NBINS], F32)
        sume = singles.tile([P, NT], F32)
        for t in range(NT):
            nc.scalar.activation(et[:, t, :], lgt[:, t, :], AF.Exp,
                                 bias=negm[:, t:t + 1],
                                 accum_out=sume[:, t:t + 1])
        lse = singles.tile([P, NT], F32)
        nc.scalar.activation(lse, sume, AF.Ln)
        lzm = singles.tile([P, NT], F32)
        nc.vector.tensor_add(lzm, lse, mcol)     # logsumexp = m + log(sum exp)

        iol = singles.tile([P, NBINS], F32)   # [0, 1, ..., 49]
        ioh = singles.tile([P, NBINS], F32)   # [1, ..., 49, 3e8] (open top bin)
        iob_ap = iob.ap()
        nc.sync.dma_start(out=iol, in_=bass.AP(
            tensor=iob_ap.tensor, offset=0, ap=[[0, P]] + iob_ap.ap[1:]))
        nc.sync.dma_start(out=ioh, in_=bass.AP(
            tensor=iob_ap.tensor, offset=NBINS, ap=[[0, P]] + iob_ap.ap[1:]))
        xs_ = singles.tile([P, NT], F32)
        for t in range(NT):
            a = work1.tile([P, NBINS], F32, tag="u")
            nc.vector.tensor_scalar(a, iol, denc[:, t:t + 1], None, OP.mult)
            w1 = work1.tile([P, NBINS], F32, tag="w1")
            nc.vector.tensor_scalar(w1, a, n50[:, t:t + 1], None, OP.is_le)
            bb = work1.tile([P, NBINS], F32, tag="bb")
            nc.vector.tensor_scalar(bb, ioh, denc[:, t:t + 1], None, OP.mult)
            w2 = work1.tile([P, NBINS], F32, tag="w2")
            nc.vector.scalar_tensor_tensor(w2, bb, n50[:, t:t + 1], w1,
                                           OP.is_gt, OP.mult)
            jk2 = work1.tile([P, NBINS], F32, tag="jk2")
            # out = (1.0*w2)*logits ; accum_out = sum -> x[bin]
            nc.vector.scalar_tensor_tensor(jk2, w2, 1.0, lgt[:, t, :],
                                           OP.mult, OP.mult,
                                           accum_out=xs_[:, t:t + 1])

        pl = singles.tile([P, NT], F32)
        nc.vector.tensor_sub(pl, lzm, xs_)       # -logp[bin] = logZ - x[bin]
        lo = singles.tile([P, NT], F32)
        nc.vector.tensor_mul(lo, pl, rv)

        red = singles.tile([P, 2], F32)
        nc.vector.tensor_reduce(red[:, 0:1], lo, mybir.AxisListType.X, OP.add)
        nc.vector.tensor_reduce(red[:, 1:2], rv, mybir.AxisListType.X, OP.add)
        ones = singles.tile([P, 1], F32)
        nc.vector.memset(ones, 1.0)
        ps = psum.tile([1, 2], F32, tag="pt")   # reuse a freed pair-psum slot
        nc.tensor.matmul(ps, ones, red, start=True, stop=True)
        res = singles.tile([1, 2], F32)
        nc.vector.tensor_copy(res, ps)
        nc.sync.dma_start(out=out[:, :], in_=res)

    if split_waits:
        split_excess_waits(nc)
    return nc


def _get_nc():
    global _NC_CACHE
    if _NC_CACHE is None:
        _NC_CACHE = build_nc()
    return _NC_CACHE


def _iob_const():
    iob = np.empty((2, NBINS), np.float32)
    iob[0] = np.arange(NBINS)
    iob[1] = np.arange(1, NBINS + 1)
    iob[1, -1] = 3e8     # top bin absorbs everything (the reference clip)
    return iob


def _bf16_split3(v):
    """v (fp32/fp64) -> three bf16 terms summing to v to ~2^-26 rel."""
    a = v.astype(NPBF)
    r = v - a.astype(np.float64)
    b = r.astype(NPBF)
    c = (r - b.astype(np.float64)).astype(NPBF)
    return a, b, c


def _aug_operands(x, rows, penalty_mask=None):
    """Host-side build of the augmented [KAUG, *] bf16 matmul operands.

    x is decomposed as h+m+l (bf16 terms); bf16 products are exact on the
    PE and accumulate in fp32, so A.T @ B = n_i + n_j - 2<x~_i, x~_j>
    (+1e6 mask penalty) at fp32-level accuracy but 4x fp32 matmul speed.
    Large-magnitude rows come first so fp32 accumulation rounding happens
    before the small correction terms land."""
    h, m, low = _bf16_split3(x.astype(np.float64))
    xt_ = h.astype(np.float64) + m.astype(np.float64) + low.astype(np.float64)
    n = (xt_ ** 2).sum(-1)
    n1, n2, n3 = _bf16_split3(n)

    nr = rows.stop - rows.start
    A = np.zeros((KAUG, nr), NPBF)
    Bm = np.zeros((KAUG, x.shape[0]), NPBF)
    hT, mT, lT = h.T, m.T, low.T
    h2 = (-2.0 * h.astype(np.float32)).astype(NPBF).T
    m2 = (-2.0 * m.astype(np.float32)).astype(NPBF).T
    l2 = (-2.0 * low.astype(np.float32)).astype(NPBF).T

    A[0] = n1[rows]; A[1] = n2[rows]; A[2] = n3[rows]
    A[3:6] = h2[:, rows]
    A[6:10] = 1.0
    A[10:13] = h2[:, rows]
    A[13:16] = m2[:, rows]
    A[16:19] = h2[:, rows]
    A[19:22] = l2[:, rows]
    A[22:25] = m2[:, rows]

    Bm[0:3] = 1.0
    Bm[3:6] = hT
    Bm[6] = n1; Bm[7] = n2; Bm[8] = n3
    if penalty_mask is not None:
        Bm[9] = (np.float32(1e6) * (1.0 - penalty_mask)).astype(NPBF)
    Bm[10:13] = mT
    Bm[13:16] = hT
    Bm[16:19] = lT
    Bm[19:22] = hT
    Bm[22:25] = mT
    return np.ascontiguousarray(A), np.ascontiguousarray(Bm)


def make_in_maps(plddt_logits, x_pred, x_true, mask):
    lgf = np.ascontiguousarray(np.asarray(plddt_logits, np.float32))
    xp = np.asarray(x_pred, np.float32)
    xt = np.asarray(x_true, np.float32)
    mk = np.asarray(mask, np.float32)
    in_maps = []
    for core in range(8):
        b, h = divmod(core, 2)
        r0 = h * HALF
        rows = slice(r0, r0 + HALF)
        Amt, Bmt = _aug_operands(xt[b], rows, penalty_mask=mk[b])
        Amp, Bmp = _aug_operands(xp[b], rows)
        in_maps.append({
            "Amt": Amt, "Bmt": Bmt, "Amp": Amp, "Bmp": Bmp,
            "mcols": np.ascontiguousarray(mk[b, rows].reshape(NT, P).T),
            "lg": np.ascontiguousarray(lgf[b, rows]),
            "iob": _iob_const(),
        })
    return in_maps


def _ensure_ntff_hook():
    """Best-effort registration of the axon NTFF profiling hook so that
    trace=True works; returns True when tracing is usable."""
    try:
        from antenv.axon_hooks import get_axon_ntff_profile_hook  # noqa: F401
        return True
    except ImportError:
        pass
    try:
        import sys
        import types

        from trn_agent_boot.trn_boot import _ntff_profile_via_ctypes

        so_path = os.environ.get("PJRT_LIBRARY_PATH", "/opt/axon/libaxon_pjrt.so")
        hook = _ntff_profile_via_ctypes(so_path)
        mod = types.ModuleType("antenv.axon_hooks")
        state = {"hook": hook}
        mod.set_axon_ntff_profile_hook = lambda h: state.__setitem__("hook", h)
        mod.get_axon_ntff_profile_hook = lambda: state["hook"]
        sys.modules["antenv.axon_hooks"] = mod
        import antenv
        antenv.axon_hooks = mod
        return hook is not None
    except Exception:
        return False


def kernel(plddt_logits, x_pred, x_true, mask):
    global LAST_RESULTS
    in_maps = make_in_maps(plddt_logits, x_pred, x_true, mask)
    nc = _get_nc()
    trace = bool(os.environ.get("PLDDT_TRACE")) and _ensure_ntff_hook()
    br = run_bass_kernel_spmd(
        nc, in_maps, list(range(8)),
        trace=trace,
    )
    LAST_RESULTS = br
    tot_l = 0.0
    tot_c = 0.0
    for r in br.results:
        o = np.asarray(r["out"]).reshape(2)
        tot_l += float(o[0])
        tot_c += float(o[1])
    return np.float32(tot_l / max(tot_c, 1.0))
